# revision 4
# baseline (speedup 1.0000x reference)
"""Trainium2 Bass kernel for CombinedLoss (CrossEntropyLabelSmooth + batch-hard TripletLoss).

Contract: kernel(**inputs) takes FULL unsharded inputs (cls_score [1024,100000] f32,
global_feat [1024,768] f32, feat [1024,768] f32 (unused), labels [1024] int) and
returns (loss, id_loss, triplet_loss) as float32 scalars, matching reference.py.

Strategy (8 NeuronCores, SPMD):
  - Shard cls_score rows 128/core. Each core streams its [128, 100000] slice once
    (memory-bound term): ACT computes exp(x-SHIFT) with fused per-row accumulation
    (sumexp), DVE reduces the raw row-sums.
  - Triplet mining needs the full batch: xT=global_feat.T is replicated; each core
    computes its 128-row slice of the pairwise distance matrix on the PE (gram
    matmul augmented with a K=1 row that adds -0.5*||x_j||^2), ACT fuses
    relu(-2*psum + ||x_i||^2) = clipped squared distances, and DVE mines the
    hardest positive (mask-multiply then reduce-max) and hardest negative
    (+1e9*mask then reduce-min) in SQUARED space (sqrt is monotonic).
  - All per-row results (sumexp, rawsum, ap^2, an^2) are packed as columns of a
    [128,4] tile, transposed on the PE against an identity (exact), and shipped
    as ONE [4,128] DMA (4 descriptors). [P,1]-shaped outputs are 128 four-byte
    descriptors (~8.5us each of serialized tail) -- avoided entirely.
  - Host does the cheap scalar epilogue: score-at-label gather (128 values/core),
    log(sumexp)+SHIFT, sqrt/relu/margin, and the final means ("all-reduce").
    No Sqrt/Ln ACT tables are ever loaded on device, so the Scalar engine runs
    an uninterrupted Exp stream and the Sync engine never stalls the cls DMA
    pipeline on a mining-dependent output store.
  - The last 4000-col tile is split 4x1000 to shorten the post-stream serial
    exp/reduce chain.
"""

from contextlib import ExitStack

import numpy as np

import concourse.bass as bass
import concourse.mybir as mybir
import concourse.tile as tile
from concourse import bacc, masks
from concourse.bass_utils import run_bass_kernel_spmd

P = 128          # rows per core == SBUF partitions
N_CORES = 8
B = 1024         # batch
D = 768          # feature dim
C = 100000       # num classes
EPS = 0.1        # label smoothing
MARGIN = 0.3
SHIFT = 4.0      # exp(x - SHIFT) for headroom; added back to lse on host
BIG = 1.0e9      # mask-out constant for hardest-negative mining

F32 = mybir.dt.float32
BF16 = mybir.dt.bfloat16
AX = mybir.AxisListType
ALU = mybir.AluOpType
ACT = mybir.ActivationFunctionType


def build_program(n_classes=C, tile_f=4000, batch=B, d=D):
    """Build the per-core Bass/Tile program (same program on all cores)."""
    assert n_classes % tile_f == 0
    # Tapered tail: the Scalar engine (exp) consumes a tile only once it has
    # fully arrived, so ever-smaller final tiles keep it caught up with the
    # HBM stream and shrink the post-stream serial chain to ~1us.
    taper = [tile_f // 2] * 3 + [tile_f // 4] + [tile_f // 8] * 2
    assert sum(taper) == 2 * tile_f
    widths = [tile_f] * (n_classes // tile_f - 2) + taper
    assert sum(widths) == n_classes
    n_tiles = len(widths)
    assert d % P == 0
    kd = d // P                       # K-subtiles for the gram matmul
    assert batch % 512 == 0
    n_chunks = batch // 512           # N-chunks of the gram output

    nc = bacc.Bacc("TRN2", target_bir_lowering=False, debug=False)

    cls_d = nc.dram_tensor("cls", [P, n_classes], F32, kind="ExternalInput")
    xt_d = nc.dram_tensor("xT", [d, batch], BF16, kind="ExternalInput")
    xtc_d = nc.dram_tensor("xTc", [d, P], BF16, kind="ExternalInput")
    xc_d = nc.dram_tensor("x_core", [P, d], BF16, kind="ExternalInput")
    laball_d = nc.dram_tensor("lab_all", [1, batch], F32, kind="ExternalInput")
    labcore_d = nc.dram_tensor("lab_core", [1, P], F32, kind="ExternalInput")

    o_pack = nc.dram_tensor("o_pack", [4, P], F32, kind="ExternalOutput")

    with tile.TileContext(nc) as tc, ExitStack() as ctx:
        persist = ctx.enter_context(tc.tile_pool(name="persist", bufs=1))
        work = ctx.enter_context(tc.tile_pool(name="work", bufs=2))
        clsp = ctx.enter_context(tc.tile_pool(name="clsp", bufs=5))
        taperp = ctx.enter_context(tc.tile_pool(name="taperp", bufs=2))
        expp = ctx.enter_context(tc.tile_pool(name="expp", bufs=1))
        psum = ctx.enter_context(tc.tile_pool(name="psum", bufs=2, space="PSUM"))
        psum1 = ctx.enter_context(tc.tile_pool(name="psum1", bufs=1, space="PSUM"))

        col_off = [0]
        for w in widths:
            col_off.append(col_off[-1] + w)

        # Issue the first few cls-stream DMAs before everything else: the Sync
        # sequencer spends ~0.6us per dma_start, so putting the prologue loads
        # first would delay the HBM stream (critical path) by several us.
        n_pre = 5
        pre_tiles = []
        for i in range(n_pre):
            t = clsp.tile([P, widths[i]], F32, tag="cls_t", name=f"cls_pre{i}")
            nc.sync.dma_start(t[:], cls_d[:, col_off[i]:col_off[i + 1]])
            pre_tiles.append(t)

        # ---------------- triplet prologue: loads ----------------
        xt_tiles = []
        for k in range(kd):
            t = persist.tile([P, batch], BF16, tag=f"xt{k}")
            nc.sync.dma_start(t[:], xt_d[k * P:(k + 1) * P, :])
            xt_tiles.append(t)
        xtc_tiles = []
        for k in range(kd):
            t = persist.tile([P, P], BF16, tag=f"xtc{k}")
            nc.sync.dma_start(t[:], xtc_d[k * P:(k + 1) * P, :])
            xtc_tiles.append(t)
        xcore_t = persist.tile([P, d], BF16, tag="xcore")
        nc.sync.dma_start(xcore_t[:], xc_d[:])

        # labels as f32 rows (single-descriptor DMAs; no [P,1]-fragmented loads)
        lab_row = persist.tile([1, batch], F32, tag="lab_row")
        nc.sync.dma_start(lab_row[:], laball_d[:])
        lab_crow = persist.tile([1, P], F32, tag="lab_crow")
        nc.sync.dma_start(lab_crow[:], labcore_d[:])

        # constants (memset on gpsimd; also used as matmul broadcast vectors)
        ones_col = persist.tile([P, 1], F32, tag="ones_col")
        nc.gpsimd.memset(ones_col[:], 1.0)
        ones_row = persist.tile([1, P], F32, tag="ones_row")
        nc.gpsimd.memset(ones_row[:], 1.0)
        b_shift = persist.tile([P, 1], F32, tag="b_shift")
        nc.gpsimd.memset(b_shift[:], -SHIFT)

        # identity for the exact PE transpose of the output pack
        ident = persist.tile([P, P], F32, tag="ident")
        masks.make_identity(nc, ident[:])

        # this core's labels as a [P,1] column via exact K=1 PE transpose
        pl_lab = psum1.tile([P, 1], F32, tag="lab_t")
        nc.tensor.matmul(pl_lab[:], lhsT=lab_crow[:], rhs=ones_row[0:1, 0:1],
                         start=True, stop=True)
        lab_cf = persist.tile([P, 1], F32, tag="lab_cf")
        nc.vector.tensor_copy(lab_cf[:], pl_lab[:])

        # is_pos mask (1.0 where labels match, incl. diagonal) and BIG*mask,
        # built per 512-column chunk straight from the PSUM broadcast
        mask = persist.tile([P, batch], F32, tag="mask")
        bigm = persist.tile([P, batch], F32, tag="bigm")
        for h in range(n_chunks):
            cs = slice(h * 512, (h + 1) * 512)
            pl = psum.tile([P, 512], F32, tag="lab_bc")
            nc.tensor.matmul(pl[:], lhsT=ones_row[:], rhs=lab_row[0:1, cs],
                             start=True, stop=True)
            nc.vector.tensor_scalar(
                out=mask[:, cs], in0=pl[:], scalar1=lab_cf[:], scalar2=None,
                op0=ALU.is_equal,
            )
            nc.vector.tensor_scalar(
                out=bigm[:, cs], in0=mask[:, cs], scalar1=BIG, scalar2=None,
                op0=ALU.mult,
            )

        # ---------------- sq_j = ||x_j||^2 via PE column-sum ----------------
        psq = [psum1.tile([1, 512], F32, tag=f"psq{h}", name=f"psq{h}")
               for h in range(n_chunks)]
        for k in range(kd):
            xsq = work.tile([P, batch], F32, tag="xsq")
            nc.scalar.activation(xsq[:], xt_tiles[k][:], ACT.Square)
            for h in range(n_chunks):
                nc.tensor.matmul(
                    psq[h][:], lhsT=ones_col[:], rhs=xsq[:, h * 512:(h + 1) * 512],
                    start=(k == 0), stop=(k == kd - 1), skip_group_check=True,
                )
        # msq row = -0.5 * sq_j (feeds the K=1 augmentation matmul)
        msq = persist.tile([1, batch], F32, tag="msq")
        for h in range(n_chunks):
            nc.vector.tensor_scalar(
                out=msq[0:1, h * 512:(h + 1) * 512], in0=psq[h][:],
                scalar1=-0.5, scalar2=None, op0=ALU.mult,
            )

        # sq_i for this core's rows, via ACT Square with fused row-accumulate
        sq_core = persist.tile([P, 1], F32, tag="sq_core")
        xsq_c = work.tile([P, d], F32, tag="xsq_c")
        nc.scalar.activation(xsq_c[:], xcore_t[:], ACT.Square, accum_out=sq_core[:])

        # ---------------- gram + batch-hard mining (squared space) ----------
        ap2 = persist.tile([P, n_chunks], F32, tag="ap2")
        an2 = persist.tile([P, n_chunks], F32, tag="an2")
        for h in range(n_chunks):
            cs = slice(h * 512, (h + 1) * 512)
            pg = psum.tile([P, 512], F32, tag="gram")
            for k in range(kd):
                nc.tensor.matmul(
                    pg[:], lhsT=xtc_tiles[k][:], rhs=xt_tiles[k][:, cs],
                    start=(k == 0), stop=False,
                )
            nc.tensor.matmul(
                pg[:], lhsT=ones_row[:], rhs=msq[0:1, cs], start=False, stop=True,
            )
            # d2 = relu(-2*(dot - 0.5*sq_j) + sq_i) = clip(dist^2, 0)
            d2 = work.tile([P, 512], F32, tag="d2")
            nc.scalar.activation(d2[:], pg[:], ACT.Relu, bias=sq_core[:], scale=-2.0)
            # hardest positive (squared): max over j of d2 * mask
            scr = work.tile([P, 512], F32, tag="scr")
            nc.vector.tensor_tensor(out=scr[:], in0=d2[:], in1=mask[:, cs],
                                    op=ALU.mult)
            nc.vector.tensor_reduce(ap2[:, h:h + 1], scr[:], axis=AX.X,
                                    op=ALU.max)
            # hardest negative (squared): min over j of d2 + BIG*mask
            scr2 = work.tile([P, 512], F32, tag="scr2")
            nc.vector.tensor_tensor(out=scr2[:], in0=d2[:], in1=bigm[:, cs],
                                    op=ALU.add)
            nc.vector.tensor_reduce(an2[:, h:h + 1], scr2[:], axis=AX.X,
                                    op=ALU.min)

        # ---------------- CE stream ----------------
        esum = persist.tile([P, n_tiles], F32, tag="esum")
        rsum = persist.tile([P, n_tiles], F32, tag="rsum")
        for i in range(n_tiles):
            w = widths[i]
            if i < len(pre_tiles):
                t = pre_tiles[i]
            elif w == tile_f:
                t = clsp.tile([P, w], F32, tag="cls_t")
                nc.sync.dma_start(t[:], cls_d[:, col_off[i]:col_off[i + 1]])
            else:
                t = taperp.tile([P, w], F32, tag=f"cls_s{w}")
                nc.sync.dma_start(t[:], cls_d[:, col_off[i]:col_off[i + 1]])
            e = expp.tile([P, w], BF16, tag=f"exp_{w}")
            nc.scalar.activation(
                e[:], t[:], ACT.Exp, bias=b_shift[:], accum_out=esum[:, i:i + 1],
            )
            nc.vector.tensor_reduce(
                rsum[:, i:i + 1], t[:], axis=AX.X, op=ALU.add,
            )

        # ---------------- pack per-row results and ship one [4,P] DMA -------
        pack = persist.tile([P, 4], F32, tag="pack")
        nc.vector.tensor_reduce(pack[:, 0:1], esum[:, 0:n_tiles], axis=AX.X,
                                op=ALU.add)
        nc.vector.tensor_reduce(pack[:, 1:2], rsum[:, 0:n_tiles], axis=AX.X,
                                op=ALU.add)
        nc.vector.tensor_reduce(pack[:, 2:3], ap2[:, 0:n_chunks], axis=AX.X,
                                op=ALU.max)
        nc.vector.tensor_reduce(pack[:, 3:4], an2[:, 0:n_chunks], axis=AX.X,
                                op=ALU.min)

        pt = psum1.tile([4, P], F32, tag="packT")
        nc.tensor.transpose(pt[:], pack[:], ident[:])
        packT = persist.tile([4, P], F32, tag="packT_s")
        nc.vector.tensor_copy(packT[:], pt[:])
        nc.sync.dma_start(o_pack[:], packT[:])

    nc.compile()
    return nc


_CACHE = {}
LAST_RESULTS = None


def _get_program(n_classes, batch, d):
    key = (n_classes, batch, d)
    if key not in _CACHE:
        tile_f = 4000 if n_classes % 4000 == 0 else n_classes // 4
        _CACHE[key] = build_program(n_classes=n_classes, tile_f=tile_f,
                                    batch=batch, d=d)
    return _CACHE[key]


def make_in_maps(cls, gf, lab):
    """Per-core input dicts (host-side sharding)."""
    import ml_dtypes
    batch = cls.shape[0]
    rows = batch // N_CORES
    gf16 = gf.astype(ml_dtypes.bfloat16)
    xt = np.ascontiguousarray(gf16.T)                    # [d, batch] bf16
    labf = lab.astype(np.float32).reshape(1, batch)
    in_maps = []
    for c in range(N_CORES):
        rs = slice(c * rows, (c + 1) * rows)
        in_maps.append({
            "cls": cls[rs],
            "xT": xt,
            "xTc": np.ascontiguousarray(xt[:, rs]),
            "x_core": gf16[rs],
            "lab_all": labf,
            "lab_core": np.ascontiguousarray(labf[:, rs]),
        })
    return in_maps


def kernel(cls_score, global_feat, feat, labels, trace=False):
    global LAST_RESULTS
    del feat  # unused by the forward pass (signature parity with reference)

    cls = np.ascontiguousarray(np.asarray(cls_score, dtype=np.float32))
    gf = np.ascontiguousarray(np.asarray(global_feat, dtype=np.float32))
    lab = np.asarray(labels).astype(np.int64)
    batch, n_classes = cls.shape
    d = gf.shape[1]
    assert batch % N_CORES == 0
    rows = batch // N_CORES
    assert rows == P, f"expected {P} rows/core, got {rows}"

    nc = _get_program(n_classes, batch, d)
    in_maps = make_in_maps(cls, gf, lab)

    res = run_bass_kernel_spmd(nc, in_maps, core_ids=list(range(N_CORES)),
                               trace=trace)
    LAST_RESULTS = res

    packs = [np.asarray(r["o_pack"], dtype=np.float64) for r in res.results]
    sumexp = np.concatenate([p[0] for p in packs])
    raw = np.concatenate([p[1] for p in packs])
    ap2 = np.concatenate([p[2] for p in packs])
    an2 = np.concatenate([p[3] for p in packs])

    # host scalar epilogue (cheap): gather, log, sqrt/relu, means
    lse = np.log(sumexp) + SHIFT
    sy = cls[np.arange(batch), lab].astype(np.float64)
    contrib = (1.0 - EPS) * sy + (EPS / n_classes) * raw - lse
    id_loss = -np.mean(contrib)
    ap = np.sqrt(np.clip(ap2, 1e-12, None))
    an = np.sqrt(np.clip(an2, 1e-12, None))
    triplet_loss = np.mean(np.maximum(ap - an + MARGIN, 0.0))
    loss = id_loss + triplet_loss
    return (np.float32(loss), np.float32(id_loss), np.float32(triplet_loss))


# revision 5
# speedup vs baseline: 1.0230x; 1.0230x over previous
"""Trainium2 Bass kernel for CombinedLoss (CrossEntropyLabelSmooth + batch-hard TripletLoss).

Contract: kernel(**inputs) takes FULL unsharded inputs (cls_score [1024,100000] f32,
global_feat [1024,768] f32, feat [1024,768] f32 (unused), labels [1024] int) and
returns (loss, id_loss, triplet_loss) as float32 scalars, matching reference.py.

Strategy (8 NeuronCores, SPMD):
  - Shard cls_score rows 128/core. Each core streams its [128, 100000] slice once
    (memory-bound term): ACT computes exp(x-SHIFT) with fused per-row accumulation
    (sumexp), DVE reduces the raw row-sums.
  - Triplet mining needs the full batch: xT=global_feat.T is replicated; each core
    computes its 128-row slice of the pairwise distance matrix on the PE (gram
    matmul augmented with a K=1 row that adds -0.5*||x_j||^2), ACT fuses
    relu(-2*psum + ||x_i||^2) = clipped squared distances, and DVE mines the
    hardest positive (mask-multiply then reduce-max) and hardest negative
    (+1e9*mask then reduce-min) in SQUARED space (sqrt is monotonic).
  - All per-row results (sumexp, rawsum, ap^2, an^2) are packed as columns of a
    [128,4] tile, transposed on the PE against an identity (exact), and shipped
    as ONE [4,128] DMA (4 descriptors). [P,1]-shaped outputs are 128 four-byte
    descriptors (~8.5us each of serialized tail) -- avoided entirely.
  - Host does the cheap scalar epilogue: score-at-label gather (128 values/core),
    log(sumexp)+SHIFT, sqrt/relu/margin, and the final means ("all-reduce").
    No Sqrt/Ln ACT tables are ever loaded on device, so the Scalar engine runs
    an uninterrupted Exp stream and the Sync engine never stalls the cls DMA
    pipeline on a mining-dependent output store.
  - The last 4000-col tile is split 4x1000 to shorten the post-stream serial
    exp/reduce chain.
"""

from contextlib import ExitStack

import numpy as np

import concourse.bass as bass
import concourse.mybir as mybir
import concourse.tile as tile
from concourse import bacc, masks
from concourse.bass_utils import run_bass_kernel_spmd

P = 128          # rows per core == SBUF partitions
N_CORES = 8
B = 1024         # batch
D = 768          # feature dim
C = 100000       # num classes
EPS = 0.1        # label smoothing
MARGIN = 0.3
SHIFT = 4.0      # exp(x - SHIFT) for headroom; added back to lse on host
BIG = 1.0e9      # mask-out constant for hardest-negative mining

F32 = mybir.dt.float32
BF16 = mybir.dt.bfloat16
AX = mybir.AxisListType
ALU = mybir.AluOpType
ACT = mybir.ActivationFunctionType


def build_program(n_classes=C, tile_f=4000, batch=B, d=D):
    """Build the per-core Bass/Tile program (same program on all cores)."""
    assert n_classes % tile_f == 0
    # Tapered tail: the Scalar engine (exp) consumes a tile only once it has
    # fully arrived, so ever-smaller final tiles keep it caught up with the
    # HBM stream and shrink the post-stream serial chain to ~1us.
    taper = [tile_f // 2] * 3 + [tile_f // 4] + [tile_f // 8] * 2
    assert sum(taper) == 2 * tile_f
    widths = [tile_f] * (n_classes // tile_f - 2) + taper
    assert sum(widths) == n_classes
    n_tiles = len(widths)
    assert d % P == 0
    kd = d // P                       # K-subtiles for the gram matmul
    assert batch % 512 == 0
    n_chunks = batch // 512           # N-chunks of the gram output

    nc = bacc.Bacc("TRN2", target_bir_lowering=False, debug=False)

    cls_d = nc.dram_tensor("cls", [P, n_classes], F32, kind="ExternalInput")
    xt_d = nc.dram_tensor("xT", [d, batch], BF16, kind="ExternalInput")
    xtc_d = nc.dram_tensor("xTc", [d, P], BF16, kind="ExternalInput")
    xc_d = nc.dram_tensor("x_core", [P, d], BF16, kind="ExternalInput")
    laball_d = nc.dram_tensor("lab_all", [1, batch], F32, kind="ExternalInput")
    labcore_d = nc.dram_tensor("lab_core", [1, P], F32, kind="ExternalInput")

    o_pack = nc.dram_tensor("o_pack", [4, P], F32, kind="ExternalOutput")

    with tile.TileContext(nc) as tc, ExitStack() as ctx:
        persist = ctx.enter_context(tc.tile_pool(name="persist", bufs=1))
        work = ctx.enter_context(tc.tile_pool(name="work", bufs=2))
        clsp = ctx.enter_context(tc.tile_pool(name="clsp", bufs=5))
        taperp = ctx.enter_context(tc.tile_pool(name="taperp", bufs=3))
        expp = ctx.enter_context(tc.tile_pool(name="expp", bufs=1))
        psum = ctx.enter_context(tc.tile_pool(name="psum", bufs=2, space="PSUM"))
        psum1 = ctx.enter_context(tc.tile_pool(name="psum1", bufs=1, space="PSUM"))

        col_off = [0]
        for w in widths:
            col_off.append(col_off[-1] + w)

        # Issue the first few cls-stream DMAs before everything else: the Sync
        # sequencer spends ~0.6us per dma_start, so putting the prologue loads
        # first would delay the HBM stream (critical path) by several us.
        n_pre = 5
        pre_tiles = []
        for i in range(n_pre):
            t = clsp.tile([P, widths[i]], F32, tag="cls_t", name=f"cls_pre{i}")
            nc.sync.dma_start(t[:], cls_d[:, col_off[i]:col_off[i + 1]])
            pre_tiles.append(t)

        # ---------------- triplet prologue: loads ----------------
        xt_tiles = []
        for k in range(kd):
            t = persist.tile([P, batch], BF16, tag=f"xt{k}")
            nc.scalar.dma_start(t[:], xt_d[k * P:(k + 1) * P, :])
            xt_tiles.append(t)
        xtc_tiles = []
        for k in range(kd):
            t = persist.tile([P, P], BF16, tag=f"xtc{k}")
            nc.scalar.dma_start(t[:], xtc_d[k * P:(k + 1) * P, :])
            xtc_tiles.append(t)
        xcore_t = persist.tile([P, d], BF16, tag="xcore")
        nc.scalar.dma_start(xcore_t[:], xc_d[:])

        # labels as f32 rows (single-descriptor DMAs; no [P,1]-fragmented loads)
        lab_row = persist.tile([1, batch], F32, tag="lab_row")
        nc.scalar.dma_start(lab_row[:], laball_d[:])
        lab_crow = persist.tile([1, P], F32, tag="lab_crow")
        nc.scalar.dma_start(lab_crow[:], labcore_d[:])

        # constants (memset on gpsimd; also used as matmul broadcast vectors)
        ones_col = persist.tile([P, 1], F32, tag="ones_col")
        nc.gpsimd.memset(ones_col[:], 1.0)
        ones_row = persist.tile([1, P], F32, tag="ones_row")
        nc.gpsimd.memset(ones_row[:], 1.0)
        b_shift = persist.tile([P, 1], F32, tag="b_shift")
        nc.gpsimd.memset(b_shift[:], -SHIFT)

        # identity for the exact PE transpose of the output pack
        ident = persist.tile([P, P], F32, tag="ident")
        masks.make_identity(nc, ident[:])

        # this core's labels as a [P,1] column via exact K=1 PE transpose
        pl_lab = psum1.tile([P, 1], F32, tag="lab_t")
        nc.tensor.matmul(pl_lab[:], lhsT=lab_crow[:], rhs=ones_row[0:1, 0:1],
                         start=True, stop=True)
        lab_cf = persist.tile([P, 1], F32, tag="lab_cf")
        nc.vector.tensor_copy(lab_cf[:], pl_lab[:])

        # is_pos mask (1.0 where labels match, incl. diagonal) and BIG*mask,
        # built per 512-column chunk straight from the PSUM broadcast
        mask = persist.tile([P, batch], F32, tag="mask")
        bigm = persist.tile([P, batch], F32, tag="bigm")
        for h in range(n_chunks):
            cs = slice(h * 512, (h + 1) * 512)
            pl = psum.tile([P, 512], F32, tag="lab_bc")
            nc.tensor.matmul(pl[:], lhsT=ones_row[:], rhs=lab_row[0:1, cs],
                             start=True, stop=True)
            nc.vector.tensor_scalar(
                out=mask[:, cs], in0=pl[:], scalar1=lab_cf[:], scalar2=None,
                op0=ALU.is_equal,
            )
            nc.vector.tensor_scalar(
                out=bigm[:, cs], in0=mask[:, cs], scalar1=BIG, scalar2=None,
                op0=ALU.mult,
            )

        # ---------------- sq_j = ||x_j||^2 via PE column-sum ----------------
        psq = [psum1.tile([1, 512], F32, tag=f"psq{h}", name=f"psq{h}")
               for h in range(n_chunks)]
        for k in range(kd):
            xsq = work.tile([P, batch], F32, tag="xsq")
            nc.scalar.activation(xsq[:], xt_tiles[k][:], ACT.Square)
            for h in range(n_chunks):
                nc.tensor.matmul(
                    psq[h][:], lhsT=ones_col[:], rhs=xsq[:, h * 512:(h + 1) * 512],
                    start=(k == 0), stop=(k == kd - 1), skip_group_check=True,
                )
        # msq row = -0.5 * sq_j (feeds the K=1 augmentation matmul)
        msq = persist.tile([1, batch], F32, tag="msq")
        for h in range(n_chunks):
            nc.vector.tensor_scalar(
                out=msq[0:1, h * 512:(h + 1) * 512], in0=psq[h][:],
                scalar1=-0.5, scalar2=None, op0=ALU.mult,
            )

        # sq_i for this core's rows, via ACT Square with fused row-accumulate
        sq_core = persist.tile([P, 1], F32, tag="sq_core")
        xsq_c = work.tile([P, d], F32, tag="xsq_c")
        nc.scalar.activation(xsq_c[:], xcore_t[:], ACT.Square, accum_out=sq_core[:])

        # ---------------- gram + batch-hard mining (squared space) ----------
        ap2 = persist.tile([P, n_chunks], F32, tag="ap2")
        an2 = persist.tile([P, n_chunks], F32, tag="an2")
        for h in range(n_chunks):
            cs = slice(h * 512, (h + 1) * 512)
            pg = psum.tile([P, 512], F32, tag="gram")
            for k in range(kd):
                nc.tensor.matmul(
                    pg[:], lhsT=xtc_tiles[k][:], rhs=xt_tiles[k][:, cs],
                    start=(k == 0), stop=False,
                )
            nc.tensor.matmul(
                pg[:], lhsT=ones_row[:], rhs=msq[0:1, cs], start=False, stop=True,
            )
            # d2 = relu(-2*(dot - 0.5*sq_j) + sq_i) = clip(dist^2, 0)
            d2 = work.tile([P, 512], F32, tag="d2")
            nc.scalar.activation(d2[:], pg[:], ACT.Relu, bias=sq_core[:], scale=-2.0)
            # hardest positive (squared): max over j of d2 * mask
            scr = work.tile([P, 512], F32, tag="scr")
            nc.vector.tensor_tensor(out=scr[:], in0=d2[:], in1=mask[:, cs],
                                    op=ALU.mult)
            nc.vector.tensor_reduce(ap2[:, h:h + 1], scr[:], axis=AX.X,
                                    op=ALU.max)
            # hardest negative (squared): min over j of d2 + BIG*mask
            scr2 = work.tile([P, 512], F32, tag="scr2")
            nc.vector.tensor_tensor(out=scr2[:], in0=d2[:], in1=bigm[:, cs],
                                    op=ALU.add)
            nc.vector.tensor_reduce(an2[:, h:h + 1], scr2[:], axis=AX.X,
                                    op=ALU.min)

        # ---------------- CE stream ----------------
        esum = persist.tile([P, n_tiles], F32, tag="esum")
        rsum = persist.tile([P, n_tiles], F32, tag="rsum")
        for i in range(n_tiles):
            w = widths[i]
            if i < len(pre_tiles):
                t = pre_tiles[i]
            elif w == tile_f:
                t = clsp.tile([P, w], F32, tag="cls_t")
                nc.sync.dma_start(t[:], cls_d[:, col_off[i]:col_off[i + 1]])
            else:
                t = taperp.tile([P, w], F32, tag=f"cls_s{w}")
                nc.sync.dma_start(t[:], cls_d[:, col_off[i]:col_off[i + 1]])
            e = expp.tile([P, w], BF16, tag=f"exp_{w}")
            nc.scalar.activation(
                e[:], t[:], ACT.Exp, bias=b_shift[:], accum_out=esum[:, i:i + 1],
            )
            nc.vector.tensor_reduce(
                rsum[:, i:i + 1], t[:], axis=AX.X, op=ALU.add,
            )

        # ---------------- pack per-row results and ship one [4,P] DMA -------
        pack = persist.tile([P, 4], F32, tag="pack")
        nc.vector.tensor_reduce(pack[:, 0:1], esum[:, 0:n_tiles], axis=AX.X,
                                op=ALU.add)
        nc.vector.tensor_reduce(pack[:, 1:2], rsum[:, 0:n_tiles], axis=AX.X,
                                op=ALU.add)
        nc.vector.tensor_reduce(pack[:, 2:3], ap2[:, 0:n_chunks], axis=AX.X,
                                op=ALU.max)
        nc.vector.tensor_reduce(pack[:, 3:4], an2[:, 0:n_chunks], axis=AX.X,
                                op=ALU.min)

        pt = psum1.tile([4, P], F32, tag="packT")
        nc.tensor.transpose(pt[:], pack[:], ident[:])
        packT = persist.tile([4, P], F32, tag="packT_s")
        nc.vector.tensor_copy(packT[:], pt[:])
        nc.sync.dma_start(o_pack[:], packT[:])

    nc.compile()
    return nc


_CACHE = {}
LAST_RESULTS = None


def _get_program(n_classes, batch, d):
    key = (n_classes, batch, d)
    if key not in _CACHE:
        tile_f = 4000 if n_classes % 4000 == 0 else n_classes // 4
        _CACHE[key] = build_program(n_classes=n_classes, tile_f=tile_f,
                                    batch=batch, d=d)
    return _CACHE[key]


def make_in_maps(cls, gf, lab):
    """Per-core input dicts (host-side sharding)."""
    import ml_dtypes
    batch = cls.shape[0]
    rows = batch // N_CORES
    gf16 = gf.astype(ml_dtypes.bfloat16)
    xt = np.ascontiguousarray(gf16.T)                    # [d, batch] bf16
    labf = lab.astype(np.float32).reshape(1, batch)
    in_maps = []
    for c in range(N_CORES):
        rs = slice(c * rows, (c + 1) * rows)
        in_maps.append({
            "cls": cls[rs],
            "xT": xt,
            "xTc": np.ascontiguousarray(xt[:, rs]),
            "x_core": gf16[rs],
            "lab_all": labf,
            "lab_core": np.ascontiguousarray(labf[:, rs]),
        })
    return in_maps


def kernel(cls_score, global_feat, feat, labels, trace=False):
    global LAST_RESULTS
    del feat  # unused by the forward pass (signature parity with reference)

    cls = np.ascontiguousarray(np.asarray(cls_score, dtype=np.float32))
    gf = np.ascontiguousarray(np.asarray(global_feat, dtype=np.float32))
    lab = np.asarray(labels).astype(np.int64)
    batch, n_classes = cls.shape
    d = gf.shape[1]
    assert batch % N_CORES == 0
    rows = batch // N_CORES
    assert rows == P, f"expected {P} rows/core, got {rows}"

    nc = _get_program(n_classes, batch, d)
    in_maps = make_in_maps(cls, gf, lab)

    res = run_bass_kernel_spmd(nc, in_maps, core_ids=list(range(N_CORES)),
                               trace=trace)
    LAST_RESULTS = res

    packs = [np.asarray(r["o_pack"], dtype=np.float64) for r in res.results]
    sumexp = np.concatenate([p[0] for p in packs])
    raw = np.concatenate([p[1] for p in packs])
    ap2 = np.concatenate([p[2] for p in packs])
    an2 = np.concatenate([p[3] for p in packs])

    # host scalar epilogue (cheap): gather, log, sqrt/relu, means
    lse = np.log(sumexp) + SHIFT
    sy = cls[np.arange(batch), lab].astype(np.float64)
    contrib = (1.0 - EPS) * sy + (EPS / n_classes) * raw - lse
    id_loss = -np.mean(contrib)
    ap = np.sqrt(np.clip(ap2, 1e-12, None))
    an = np.sqrt(np.clip(an2, 1e-12, None))
    triplet_loss = np.mean(np.maximum(ap - an + MARGIN, 0.0))
    loss = id_loss + triplet_loss
    return (np.float32(loss), np.float32(id_loss), np.float32(triplet_loss))
